# revision 40
# baseline (speedup 1.0000x reference)
"""Trainium2 Bass kernel for nn_BatchHighOrderActivation.

Reference semantics (per batch b, channel g):
    sort the ARITY=4 values x = X[b,g,:], build barycentric coefficients from
    the sorted gaps, gather params rows by reverse-cumsum bitmasks, contract.

Sort/gather-free reformulation (multilinear simplex / Lovasz form):
    out[b,g,:] = sum_{m=1..15} w[b,g,m] * params[g,m,:]
    w[m] = relu( min_{i in m} x_i - max_{i not in m} x_i )   for m != 15
    w[15] = min_i x_i                                        (no relu)

Kernel structure per core (batch-sharded, 512 rows/core), fp16 internal
compute (inputs/outputs fp32; absmax-relative error ~5e-4):
  - ACT : cast X into 4 contiguous fp16 planes (enables DVE 2x packing)
  - DVE : subset min/max tree + 14 subtractions -> W [128b, (q,m,gl)] fp16,
          relu, m15 column
  - PE  : transpose W 128x128 chunks via fp16 identity matmul
  - DVE : PSUM->SBUF W^T evacuation
  - PE  : block-diagonal fp16 matmul (8 channels/group, K=(m,gl)=128, N=256)
          vs host-precomputed fp16 block-diag params table, fp32 PSUM
  - DVE/ACT: PSUM->SBUF output staging, 8KB-contiguous DMA to HBM
"""

import numpy as np
from contextlib import ExitStack

import concourse.bass as bass
import concourse.mybir as mybir
import concourse.tile as tile
from concourse import bacc
from concourse.bass_utils import run_bass_kernel_spmd

F32 = mybir.dt.float32
F16 = mybir.dt.float16
NCORES = 8
B, G, A, O = 4096, 512, 4, 32
BS = B // NCORES        # 512 batch rows per core
NBT = BS // 128         # 4 b-tiles per core
NQ = G // 8             # 64 channel groups of 8

_PAIRS = [(0, 1), (0, 2), (0, 3), (1, 2), (1, 3), (2, 3)]
_TRIPLES = [(0, 1, 2), (0, 1, 3), (0, 2, 3), (1, 2, 3)]

_cached_nc = None


def _build_program():
    nc = bacc.Bacc("TRN2", target_bir_lowering=False, debug=False, num_devices=NCORES)

    # X pre-deinterleaved on host into fp16 planes: [BS, A, G]
    x_d = nc.dram_tensor("x", [BS, A * G], F16, kind="ExternalInput").ap()
    pbd_d = nc.dram_tensor("pbd", [128, NQ * 256], F16, kind="ExternalInput").ap()
    id_d = nc.dram_tensor("ident", [128, 128], F16, kind="ExternalInput").ap()
    out_d = nc.dram_tensor("out", [BS, G * O], F32, kind="ExternalOutput").ap()

    with ExitStack() as ctx:
        tc = ctx.enter_context(tile.TileContext(nc))
        persist = ctx.enter_context(tc.tile_pool(name="persist", bufs=1))
        plpool = ctx.enter_context(tc.tile_pool(name="pl", bufs=3))
        treep = ctx.enter_context(tc.tile_pool(name="tree", bufs=2))
        wpool = ctx.enter_context(tc.tile_pool(name="w", bufs=2))
        lhsp = ctx.enter_context(tc.tile_pool(name="lt", bufs=4))
        stgp = ctx.enter_context(tc.tile_pool(name="stg", bufs=3))
        ptp = ctx.enter_context(tc.tile_pool(name="pt", bufs=2, space="PSUM"))
        pmp = ctx.enter_context(tc.tile_pool(name="pm", bufs=3, space="PSUM"))

        # first input loads dispatch before anything else occupies gpsimd;
        # all input loads go through SWDGE (gpsimd) queues so they are not
        # stuck behind the output-store backlog on the HWDGE queues
        pbd = persist.tile([128, NQ * 256], F16)
        pl0 = plpool.tile([128, A, G], F16, tag="pl")
        identity = persist.tile([128, 128], F16)
        nc.gpsimd.dma_start(pl0[:].rearrange("p a g -> p (a g)"), x_d[0:128, :])
        nc.gpsimd.dma_start(identity[:], id_d[:])
        nc.gpsimd.dma_start(pbd[:], pbd_d[:])

        for bt in range(NBT):
            if bt == 0:
                pl = pl0
            else:
                pl = plpool.tile([128, A, G], F16, tag="pl")
                nc.gpsimd.dma_start(
                    pl[:].rearrange("p a g -> p (a g)"), x_d[bt * 128:(bt + 1) * 128, :]
                )
            s = [pl[:, i, :] for i in range(A)]

            tr = treep.tile([128, 21, G], F16, tag="tree")
            slot = [0]
            mn, mx = {}, {}

            def alloc():
                ap = tr[:, slot[0], :]
                slot[0] += 1
                return ap

            for (i, j) in _PAIRS:
                mn[(i, j)] = alloc()
                nc.vector.tensor_tensor(mn[(i, j)], s[i], s[j], mybir.AluOpType.min)
            for (i, j) in _PAIRS:
                mx[(i, j)] = alloc()
                nc.vector.tensor_tensor(mx[(i, j)], s[i], s[j], mybir.AluOpType.max)
            for (i, j, k) in _TRIPLES:
                mn[(i, j, k)] = alloc()
                nc.vector.tensor_tensor(mn[(i, j, k)], mn[(i, j)], s[k], mybir.AluOpType.min)
            for (i, j, k) in _TRIPLES:
                mx[(i, j, k)] = alloc()
                nc.vector.tensor_tensor(mx[(i, j, k)], mx[(i, j)], s[k], mybir.AluOpType.max)
            mn[(0, 1, 2, 3)] = alloc()
            nc.vector.tensor_tensor(mn[(0, 1, 2, 3)], mn[(0, 1, 2)], s[3], mybir.AluOpType.min)

            def sub_ap(S):
                return s[S[0]] if len(S) == 1 else mn[S]

            def sup_ap(Cm):
                return s[Cm[0]] if len(Cm) == 1 else mx[Cm]

            # W layout: free = q*128 + m*8 + gl (K-order (m,gl)); sub writes
            # are 8-element step-1 runs, transpose inputs are contiguous.
            # Processed in two q-halves so PE/evac of half 0 overlaps the
            # DVE work of half 1.
            wt = wpool.tile([128, NQ * 128], F16, tag="w")
            wv4 = wt.rearrange("p (q m gl) -> p q m gl", m=16, gl=8)
            wvr = wt.rearrange("p (q r) -> p q r", r=128)
            ev = [0]
            oev = [0]
            for hf in range(2):
                qh = slice(hf * 32, hf * 32 + 32)
                gh = slice(hf * 256, hf * 256 + 256)
                # m=0 columns: never written otherwise; must be finite (their
                # matmul contribution is zeroed by the zero params rows).
                nc.gpsimd.memset(wv4[:, qh, 0, :], 0.0)
                for m in range(1, 15):
                    S = tuple(i for i in range(A) if (m >> i) & 1)
                    Cm = tuple(i for i in range(A) if not ((m >> i) & 1))
                    nc.vector.tensor_tensor(
                        wv4[:, qh, m, :],
                        sub_ap(S)[:, gh],
                        sup_ap(Cm)[:, gh],
                        mybir.AluOpType.subtract,
                    )
                # relu m=1..14 (112-wide step-1 runs); m=15 is not relu'd
                nc.vector.tensor_scalar_max(wvr[:, qh, 8:120], wvr[:, qh, 8:120], 0.0)
                nc.vector.tensor_copy(wv4[:, qh, 15, :], mn[(0, 1, 2, 3)][:, gh])

                for gp in range(2):
                    stg = stgp.tile([128, 16 * 256], F32, tag="stg")
                    for gqi in range(2):
                        q0 = hf * 32 + gp * 16 + gqi * 8
                        pt = ptp.tile([128, 8 * 128], F16, tag="pt")
                        for j in range(8):
                            q = q0 + j
                            nc.tensor.transpose(
                                pt[:, j * 128:(j + 1) * 128],
                                wt[:, q * 128:(q + 1) * 128],
                                identity[:],
                            )
                        lt = lhsp.tile([128, 8 * 128], F16, tag="lt")
                        # W^T evacuation: 4/8 DVE, 4/8 ACT
                        if ev[0] % 2 == 0:
                            nc.vector.tensor_copy(lt[:], pt[:])
                        else:
                            nc.scalar.copy(lt[:], pt[:])
                        ev[0] += 1
                        for half in range(2):
                            pm = pmp.tile([128, 1024], F32, tag="pm")
                            for j2 in range(4):
                                j = half * 4 + j2
                                qq = q0 + j
                                nc.tensor.matmul(
                                    pm[:, j2 * 256:(j2 + 1) * 256],
                                    lt[:, j * 128:(j + 1) * 128],
                                    pbd[:, qq * 256:(qq + 1) * 256],
                                    start=True,
                                    stop=True,
                                )
                            dst = stg[:, gqi * 2048 + half * 1024:
                                      gqi * 2048 + (half + 1) * 1024]
                            # out evacuation: mostly ACT; on the last b-tile
                            # DVE has no more tree work, so split evenly
                            dve_out = (oev[0] % 2 == 0) if bt == NBT - 1 else (
                                oev[0] % 8 == 4)
                            if dve_out:
                                nc.vector.tensor_copy(dst, pm[:])
                            else:
                                nc.scalar.copy(dst, pm[:])
                            oev[0] += 1
                    qq0 = hf * 32 + gp * 16
                    nc.sync.dma_start(
                        out_d[bt * 128:(bt + 1) * 128, qq0 * 256:(qq0 + 16) * 256],
                        stg[:],
                    )

    nc.compile()
    return nc


def _get_program():
    global _cached_nc
    if _cached_nc is None:
        _cached_nc = _build_program()
    return _cached_nc


def _make_inputs(X, params):
    X = np.ascontiguousarray(X, dtype=np.float32)
    params = np.ascontiguousarray(params, dtype=np.float32)
    P4 = params.reshape(NQ, 8, 16, O)                 # [q, gl, m, o]
    # block-diag table: pbd[m*8+gl, q*256 + gl*32 + o] = params[8q+gl, m, o]
    Pb = np.zeros((16, 8, NQ, 8, O), np.float32)
    for gl in range(8):
        Pb[1:, gl, :, gl, :] = P4[:, gl, 1:, :].transpose(1, 0, 2)
    pbd = np.ascontiguousarray(Pb.reshape(128, NQ * 256).astype(np.float16))
    # de-interleave X to per-arity fp16 planes: [B, G, A] -> [B, A, G]
    Xp = np.ascontiguousarray(
        X.reshape(B, G, A).transpose(0, 2, 1).astype(np.float16).reshape(B, A * G)
    )
    ident = np.eye(128, dtype=np.float16)
    in_maps = [
        {"x": Xp[c * BS:(c + 1) * BS], "pbd": pbd, "ident": ident}
        for c in range(NCORES)
    ]
    return in_maps


def kernel(X, params):
    nc = _get_program()
    in_maps = _make_inputs(X, params)
    res = run_bass_kernel_spmd(nc, in_maps, list(range(NCORES))).results
    out = np.concatenate(
        [res[c]["out"].reshape(BS, G, O) for c in range(NCORES)], axis=0
    )
    return out


def kernel_traced(X, params):
    """Like kernel() but also returns the BassKernelResults (profile info)."""
    nc = _get_program()
    in_maps = _make_inputs(X, params)
    br = run_bass_kernel_spmd(nc, in_maps, list(range(NCORES)), trace=True)
    out = np.concatenate(
        [br.results[c]["out"].reshape(BS, G, O) for c in range(NCORES)], axis=0
    )
    return out, br


# revision 43
# speedup vs baseline: 1.0818x; 1.0818x over previous
"""Trainium2 Bass kernel for nn_BatchHighOrderActivation.

Reference semantics (per batch b, channel g):
    sort the ARITY=4 values x = X[b,g,:], build barycentric coefficients from
    the sorted gaps, gather params rows by reverse-cumsum bitmasks, contract.

Sort/gather-free reformulation (multilinear simplex / Lovasz form):
    out[b,g,:] = sum_{m=1..15} w[b,g,m] * params[g,m,:]
    w[m] = relu( min_{i in m} x_i - max_{i not in m} x_i )   for m != 15
    w[15] = min_i x_i                                        (no relu)

Kernel structure per core (batch-sharded, 512 rows/core), fp16 internal
compute (inputs/outputs fp32; absmax-relative error ~5e-4):
  - ACT : cast X into 4 contiguous fp16 planes (enables DVE 2x packing)
  - DVE : subset min/max tree + 14 subtractions -> W [128b, (q,m,gl)] fp16,
          relu, m15 column
  - PE  : transpose W 128x128 chunks via fp16 identity matmul
  - DVE : PSUM->SBUF W^T evacuation
  - PE  : block-diagonal fp16 matmul (8 channels/group, K=(m,gl)=128, N=256)
          vs host-precomputed fp16 block-diag params table, fp32 PSUM
  - DVE/ACT: PSUM->SBUF output staging, 8KB-contiguous DMA to HBM
"""

import numpy as np
from contextlib import ExitStack

import concourse.bass as bass
import concourse.mybir as mybir
import concourse.tile as tile
from concourse import bacc
from concourse.bass_utils import run_bass_kernel_spmd
from concourse.masks import make_identity

F32 = mybir.dt.float32
F16 = mybir.dt.float16
NCORES = 8
B, G, A, O = 4096, 512, 4, 32
BS = B // NCORES        # 512 batch rows per core
NBT = BS // 128         # 4 b-tiles per core
NQ = G // 8             # 64 channel groups of 8

_PAIRS = [(0, 1), (0, 2), (0, 3), (1, 2), (1, 3), (2, 3)]
_TRIPLES = [(0, 1, 2), (0, 1, 3), (0, 2, 3), (1, 2, 3)]

_cached_nc = None


def _build_program():
    nc = bacc.Bacc("TRN2", target_bir_lowering=False, debug=False, num_devices=NCORES)

    # X pre-deinterleaved on host into fp16 planes: [BS, A, G]
    x_d = nc.dram_tensor("x", [BS, A * G], F16, kind="ExternalInput").ap()
    pbd_d = nc.dram_tensor("pbd", [128, NQ * 256], F16, kind="ExternalInput").ap()
    out_d = nc.dram_tensor("out", [BS, G * O], F32, kind="ExternalOutput").ap()

    with ExitStack() as ctx:
        tc = ctx.enter_context(tile.TileContext(nc))
        persist = ctx.enter_context(tc.tile_pool(name="persist", bufs=1))
        plpool = ctx.enter_context(tc.tile_pool(name="pl", bufs=3))
        treep = ctx.enter_context(tc.tile_pool(name="tree", bufs=2))
        wpool = ctx.enter_context(tc.tile_pool(name="w", bufs=2))
        lhsp = ctx.enter_context(tc.tile_pool(name="lt", bufs=4))
        stgp = ctx.enter_context(tc.tile_pool(name="stg", bufs=3))
        ptp = ctx.enter_context(tc.tile_pool(name="pt", bufs=2, space="PSUM"))
        pmp = ctx.enter_context(tc.tile_pool(name="pm", bufs=3, space="PSUM"))

        # first input loads dispatch before anything else occupies gpsimd;
        # all input loads go through SWDGE (gpsimd) queues so they are not
        # stuck behind the output-store backlog on the HWDGE queues
        pbd = persist.tile([128, NQ * 256], F16)
        pl0 = plpool.tile([128, A, G], F16, tag="pl")
        identity = persist.tile([128, 128], F16)
        nc.gpsimd.dma_start(pl0[:].rearrange("p a g -> p (a g)"), x_d[0:128, :])
        nc.gpsimd.dma_start(pbd[:], pbd_d[:])
        make_identity(nc, identity[:])

        for bt in range(NBT):
            if bt == 0:
                pl = pl0
            else:
                pl = plpool.tile([128, A, G], F16, tag="pl")
                nc.gpsimd.dma_start(
                    pl[:].rearrange("p a g -> p (a g)"), x_d[bt * 128:(bt + 1) * 128, :]
                )
            s = [pl[:, i, :] for i in range(A)]

            tr = treep.tile([128, 21, G], F16, tag="tree")
            slot = [0]
            mn, mx = {}, {}

            def alloc():
                ap = tr[:, slot[0], :]
                slot[0] += 1
                return ap

            for (i, j) in _PAIRS:
                mn[(i, j)] = alloc()
                nc.vector.tensor_tensor(mn[(i, j)], s[i], s[j], mybir.AluOpType.min)
            for (i, j) in _PAIRS:
                mx[(i, j)] = alloc()
                nc.vector.tensor_tensor(mx[(i, j)], s[i], s[j], mybir.AluOpType.max)
            for (i, j, k) in _TRIPLES:
                mn[(i, j, k)] = alloc()
                nc.vector.tensor_tensor(mn[(i, j, k)], mn[(i, j)], s[k], mybir.AluOpType.min)
            for (i, j, k) in _TRIPLES:
                mx[(i, j, k)] = alloc()
                nc.vector.tensor_tensor(mx[(i, j, k)], mx[(i, j)], s[k], mybir.AluOpType.max)
            mn[(0, 1, 2, 3)] = alloc()
            nc.vector.tensor_tensor(mn[(0, 1, 2, 3)], mn[(0, 1, 2)], s[3], mybir.AluOpType.min)

            def sub_ap(S):
                return s[S[0]] if len(S) == 1 else mn[S]

            def sup_ap(Cm):
                return s[Cm[0]] if len(Cm) == 1 else mx[Cm]

            # W layout: free = q*128 + m*8 + gl (K-order (m,gl)); sub writes
            # are 8-element step-1 runs, transpose inputs are contiguous.
            # Processed in two q-halves so PE/evac of half 0 overlaps the
            # DVE work of half 1.
            wt = wpool.tile([128, NQ * 128], F16, tag="w")
            wv4 = wt.rearrange("p (q m gl) -> p q m gl", m=16, gl=8)
            wvr = wt.rearrange("p (q r) -> p q r", r=128)
            ev = [0]
            oev = [0]
            for hf in range(2):
                qh = slice(hf * 32, hf * 32 + 32)
                gh = slice(hf * 256, hf * 256 + 256)
                # m=0 columns: never written otherwise; must be finite (their
                # matmul contribution is zeroed by the zero params rows).
                nc.gpsimd.memset(wv4[:, qh, 0, :], 0.0)
                for m in range(1, 15):
                    S = tuple(i for i in range(A) if (m >> i) & 1)
                    Cm = tuple(i for i in range(A) if not ((m >> i) & 1))
                    nc.vector.tensor_tensor(
                        wv4[:, qh, m, :],
                        sub_ap(S)[:, gh],
                        sup_ap(Cm)[:, gh],
                        mybir.AluOpType.subtract,
                    )
                # relu m=1..14 (112-wide step-1 runs); m=15 is not relu'd
                nc.vector.tensor_scalar_max(wvr[:, qh, 8:120], wvr[:, qh, 8:120], 0.0)
                nc.vector.tensor_copy(wv4[:, qh, 15, :], mn[(0, 1, 2, 3)][:, gh])

                for gp in range(2):
                    stg = stgp.tile([128, 16 * 256], F32, tag="stg")
                    for gqi in range(2):
                        q0 = hf * 32 + gp * 16 + gqi * 8
                        pt = ptp.tile([128, 8 * 128], F16, tag="pt")
                        for j in range(8):
                            q = q0 + j
                            nc.tensor.transpose(
                                pt[:, j * 128:(j + 1) * 128],
                                wt[:, q * 128:(q + 1) * 128],
                                identity[:],
                            )
                        lt = lhsp.tile([128, 8 * 128], F16, tag="lt")
                        # W^T evacuation: 4/8 DVE, 4/8 ACT
                        if ev[0] % 2 == 0:
                            nc.vector.tensor_copy(lt[:], pt[:])
                        else:
                            nc.scalar.copy(lt[:], pt[:])
                        ev[0] += 1
                        for half in range(2):
                            pm = pmp.tile([128, 1024], F32, tag="pm")
                            for j2 in range(4):
                                j = half * 4 + j2
                                qq = q0 + j
                                nc.tensor.matmul(
                                    pm[:, j2 * 256:(j2 + 1) * 256],
                                    lt[:, j * 128:(j + 1) * 128],
                                    pbd[:, qq * 256:(qq + 1) * 256],
                                    start=True,
                                    stop=True,
                                )
                            dst = stg[:, gqi * 2048 + half * 1024:
                                      gqi * 2048 + (half + 1) * 1024]
                            # out evacuation: mostly ACT; on the last b-tile
                            # DVE has no more tree work, so split evenly
                            dve_out = (oev[0] % 2 == 0) if bt == NBT - 1 else (
                                oev[0] % 8 == 4)
                            if dve_out:
                                nc.vector.tensor_copy(dst, pm[:])
                            else:
                                nc.scalar.copy(dst, pm[:])
                            oev[0] += 1
                    qq0 = hf * 32 + gp * 16
                    nc.sync.dma_start(
                        out_d[bt * 128:(bt + 1) * 128, qq0 * 256:(qq0 + 16) * 256],
                        stg[:],
                    )

    nc.compile()
    return nc


def _get_program():
    global _cached_nc
    if _cached_nc is None:
        _cached_nc = _build_program()
    return _cached_nc


def _make_inputs(X, params):
    X = np.ascontiguousarray(X, dtype=np.float32)
    params = np.ascontiguousarray(params, dtype=np.float32)
    P4 = params.reshape(NQ, 8, 16, O)                 # [q, gl, m, o]
    # block-diag table: pbd[m*8+gl, q*256 + gl*32 + o] = params[8q+gl, m, o]
    Pb = np.zeros((16, 8, NQ, 8, O), np.float32)
    for gl in range(8):
        Pb[1:, gl, :, gl, :] = P4[:, gl, 1:, :].transpose(1, 0, 2)
    pbd = np.ascontiguousarray(Pb.reshape(128, NQ * 256).astype(np.float16))
    # de-interleave X to per-arity fp16 planes: [B, G, A] -> [B, A, G]
    Xp = np.ascontiguousarray(
        X.reshape(B, G, A).transpose(0, 2, 1).astype(np.float16).reshape(B, A * G)
    )
    in_maps = [
        {"x": Xp[c * BS:(c + 1) * BS], "pbd": pbd}
        for c in range(NCORES)
    ]
    return in_maps


def kernel(X, params):
    nc = _get_program()
    in_maps = _make_inputs(X, params)
    res = run_bass_kernel_spmd(nc, in_maps, list(range(NCORES))).results
    out = np.concatenate(
        [res[c]["out"].reshape(BS, G, O) for c in range(NCORES)], axis=0
    )
    return out


def kernel_traced(X, params):
    """Like kernel() but also returns the BassKernelResults (profile info)."""
    nc = _get_program()
    in_maps = _make_inputs(X, params)
    br = run_bass_kernel_spmd(nc, in_maps, list(range(NCORES)), trace=True)
    out = np.concatenate(
        [br.results[c]["out"].reshape(BS, G, O) for c in range(NCORES)], axis=0
    )
    return out, br
